# revision 5
# baseline (speedup 1.0000x reference)
"""Trainium2 Bass kernel for AuxiliaryMultiHeadedAttention.

Reference computation (B=4, L=2048, H=256, NH=8, DH=32):
    kb   = split_heads(k_b @ Wb.T + bb)
    corr = (qh @ kh^T + qh @ kb^T) / sqrt(DH) * scale_w[h, q]
    corr = where(mask==0, -1e9, corr);  prob = softmax(corr)
    out  = merge_heads(prob @ vh) @ Ww.T + bw

Kernel strategy (8 NeuronCores):
    Shard (batch, query-half): core c -> batch c//2, queries (c%2)*1024..+1024.
    Host marshals layouts (pre-transposed bf16 k^T / k_b^T / W^T, the
    mask-interleaved V operand, scale_w chunks) so the device does zero
    layout shuffling; device computes both GEMMs, QK^T, softmax, PV and
    the output projection.
    Each core:
      keffT = (k + k_b @ Wb.T + bb)^T  [dims, keys]  bf16 (dual QK^T folded)
      qsT   = (q * scale_w/sqrt(DH))^T via DMA-xbar  [dims, queries] bf16
      S^T   = keffT_h^T @ qsT_h  (bf16 MMs, 2 heads row-tiled, fp32 psum;
              kc pairs alternate PE row groups via 64-row-shifted copies)
      P^T   = exp(S^T): split between ACT (exact exp, bf16 out) and DVE
              (Schraudolph: int16(A*x+B) bitcast to bf16, one tensor_scalar)
      PV with lhsT [m|v_h] / [v_h|m] (m = mask: masks numerator and
              denominator) -> psum rows [den0|O0|O1|den1]
      hidT  = O * recip(den)  (full-partition recip + mul, DMA realign)
      out   = hidT^T @ WwT + bw (bias via rank-1 matmul), query-half-outer
              loop lets the first half's projection overlap the second half.
    Host concatenates the 8 [1024, 256] slices.
"""

import sys

if "/opt/trn_rl_repo" not in sys.path:
    sys.path.insert(0, "/opt/trn_rl_repo")

import math

import numpy as np

B, L, H, NH, DH = 4, 2048, 256, 8, 32
LQ = 1024  # queries per core
NCORES = 8
ISQ = 1.0 / math.sqrt(DH)

# Schraudolph exp for bf16 target: bf16bits(exp(x)) ~ int16(A16*x + B16)
A16 = 128.0 / math.log(2.0)
C_OFF = 5.5
B16 = 127.0 * 128.0 - C_OFF


def _build():
    import concourse.bass as bass  # noqa: F401
    import concourse.mybir as mybir
    import concourse.tile as tile
    from concourse import bacc

    f32 = mybir.dt.float32
    i16 = mybir.dt.int16
    bf16 = mybir.dt.bfloat16
    Exp = mybir.ActivationFunctionType.Exp

    nc = bacc.Bacc("TRN2", target_bir_lowering=False, debug=False, num_devices=NCORES)

    q16_d = nc.dram_tensor("q16", [128, 2048], bf16, kind="ExternalInput")
    kT_d = nc.dram_tensor("kT", [H, L], bf16, kind="ExternalInput")
    kbT_d = nc.dram_tensor("kbT", [H, L], bf16, kind="ExternalInput")
    vmm_d = nc.dram_tensor("vmm", [128, 8192], bf16, kind="ExternalInput")
    sc8_d = nc.dram_tensor("sc8", [128, 64], f32, kind="ExternalInput")
    WbT_d = nc.dram_tensor("WbT", [H, H], bf16, kind="ExternalInput")
    WwT_d = nc.dram_tensor("WwT", [H, H], bf16, kind="ExternalInput")
    bbb_d = nc.dram_tensor("bbb", [1, H], bf16, kind="ExternalInput")
    bwb_d = nc.dram_tensor("bwb", [1, H], bf16, kind="ExternalInput")
    ones_d = nc.dram_tensor("ones", [1, L], bf16, kind="ExternalInput")
    out_d = nc.dram_tensor("out", [LQ, H], f32, kind="ExternalOutput")

    copy_flip = [0]

    with tile.TileContext(nc) as tc:
        with (
            tc.tile_pool(name="persist", bufs=1) as pp,
            tc.tile_pool(name="pt", bufs=3) as ptp,
            tc.tile_pool(name="small", bufs=2) as smp,
        ):
            # ---------------- persistent SBUF tensors ----------------
            keffT = [pp.tile([128, L], bf16, tag=f"keffT{g}", name=f"keffT{g}")
                     for g in range(2)]
            keffT2 = [pp.tile([128, L], bf16, tag=f"keffT2_{g}",
                              name=f"keffT2_{g}") for g in range(2)]
            qsT = [pp.tile([128, LQ], bf16, tag=f"qsT{g}", name=f"qsT{g}")
                   for g in range(2)]
            qsT2 = [pp.tile([128, LQ], bf16, tag=f"qsT2_{g}", name=f"qsT2_{g}")
                    for g in range(2)]
            # per (key-chunk, head): [m|v_h] (h even) / [v_h|m] (h odd);
            # m = mask column (masks numerator and denominator)
            vmm = pp.tile([128, 16 * NH * 64], bf16, tag="vmm")
            hidT = [pp.tile([128, LQ], bf16, tag=f"hidT{g}", name=f"hidT{g}")
                    for g in range(2)]
            outsb = pp.tile([128, 8 * H], f32, tag="outsb")

            kbTt = [pp.tile([128, L], bf16, tag=f"kbTt{e}", name=f"kbTt{e}")
                    for e in range(2)]
            kTt = [pp.tile([128, L], bf16, tag=f"kTt{e}", name=f"kTt{e}")
                   for e in range(2)]
            qb16 = pp.tile([128, 2048], bf16, tag="qb16")
            qs16 = [pp.tile([128, 1024], bf16, tag=f"qs16_{d}",
                            name=f"qs16_{d}") for d in range(2)]
            sc8 = pp.tile([128, 64], f32, tag="sc8")
            WbTt = [pp.tile([128, H], bf16, tag=f"WbTt{e}", name=f"WbTt{e}")
                    for e in range(2)]
            WwTt = [pp.tile([128, H], bf16, tag=f"WwTt{g}", name=f"WwTt{g}")
                    for g in range(2)]
            bbb = pp.tile([1, H], bf16, tag="bbb")
            bwb = pp.tile([1, H], bf16, tag="bwb")
            ones = pp.tile([1, L], bf16, tag="ones")

            def pcopy(dst, src):
                # alternate psum->sbuf evacuation between DVE and ACT
                if copy_flip[0] % 2 == 0:
                    nc.vector.tensor_copy(dst, src)
                else:
                    nc.scalar.copy(dst, src)
                copy_flip[0] += 1

            # ---------------- staging loads ----------------
            # gpsimd queue: kbT (keff GEMM critical path)
            for ec in range(2):
                nc.gpsimd.dma_start(out=kbTt[ec],
                                    in_=kbT_d[ec * 128:(ec + 1) * 128, :])
            # sync queue: small + q + kT (ordered by criticality)
            nc.sync.dma_start(out=WbTt[0], in_=WbT_d[0:128, :])
            nc.sync.dma_start(out=WbTt[1], in_=WbT_d[128:256, :])
            nc.sync.dma_start(out=sc8, in_=sc8_d[:, :])
            nc.sync.dma_start(out=bbb, in_=bbb_d[:, :])
            nc.sync.dma_start(out=ones, in_=ones_d[:, :])
            nc.sync.dma_start(out=qb16, in_=q16_d[:, :])
            for ec in range(2):
                nc.sync.dma_start(out=kTt[ec],
                                  in_=kT_d[ec * 128:(ec + 1) * 128, :])
            nc.sync.dma_start(out=WwTt[0], in_=WwT_d[0:128, :])
            nc.sync.dma_start(out=WwTt[1], in_=WwT_d[128:256, :])
            nc.sync.dma_start(out=bwb, in_=bwb_d[:, :])
            # scalar queue: vmm halves around the q xbar transposes
            vmv = vmm.rearrange("p (c f) -> p c f", c=2)
            nc.scalar.dma_start(out=vmv[:, 0, :], in_=vmm_d[:, 0:4096])

            # q scale: qs16[dc] = q_bf16 * (scale_w * ISQ)  [per head, query]
            sc8r = sc8.rearrange("p (c h) -> p c h", c=8)
            qv = qb16.rearrange("p (c e) -> p c e", c=8)
            for dc in range(2):
                nc.vector.tensor_mul(
                    qs16[dc].rearrange("p (c h j) -> p c h j", c=8, h=4),
                    qv[:, :, dc * 128:(dc + 1) * 128].rearrange(
                        "p c (h j) -> p c h j", h=4),
                    sc8r[:, :, dc * 4:(dc + 1) * 4][:, :, :, None].broadcast_to(
                        [128, 8, 4, 32]))
                nc.scalar.dma_start_transpose(
                    out=qsT[dc].rearrange("p (c f) -> p c f", c=8),
                    in_=qs16[dc])
                nc.sync.dma_start(out=qsT2[dc][0:64], in_=qsT[dc][64:128])
                nc.sync.dma_start(out=qsT2[dc][64:128], in_=qsT[dc][0:64])
            nc.scalar.dma_start(out=vmv[:, 1, :], in_=vmm_d[:, 4096:8192])

            # ---------------- keff GEMM ----------------
            with tc.tile_pool(name="pkeff", bufs=1, space="PSUM") as pkf:
                for dc in range(2):
                    pk = pkf.tile([128, L], f32, tag="pk", name=f"pk{dc}")
                    for ec in range(2):
                        for ns in range(4):
                            nc.tensor.matmul(
                                pk[:, ns * 512:(ns + 1) * 512],
                                lhsT=WbTt[ec][:, dc * 128:(dc + 1) * 128],
                                rhs=kbTt[ec][:, ns * 512:(ns + 1) * 512],
                                start=(ec == 0), stop=False)
                    for ns in range(4):
                        nc.tensor.matmul(
                            pk[:, ns * 512:(ns + 1) * 512],
                            lhsT=bbb[0:1, dc * 128:(dc + 1) * 128],
                            rhs=ones[0:1, ns * 512:(ns + 1) * 512],
                            start=False, stop=True)
                    # evacuate with fused +k add; then 64-row-shifted copy
                    for nh2 in range(2):
                        co = slice(nh2 * 1024, (nh2 + 1) * 1024)
                        nc.vector.tensor_add(keffT[dc][:, co], pk[:, co],
                                             kTt[dc][:, co])
                    nc.sync.dma_start(out=keffT2[dc][0:64],
                                      in_=keffT[dc][64:128])
                    nc.sync.dma_start(out=keffT2[dc][64:128],
                                      in_=keffT[dc][0:64])

            # ---------------- main attention loop ----------------
            # query-half outer so the first half's output projection can
            # overlap the second half's attention.
            # group g: heads (2g, 2g+1); chunk ch = g//2.
            # kc processed in pairs with alternating PE row groups (via the
            # 64-row-shifted tile copies): the pair's 4 QK matmuls occupy 4
            # distinct 32-row groups and stream concurrently.
            with (
                tc.tile_pool(name="pst", bufs=3, space="PSUM") as pst,
                tc.tile_pool(name="ppv", bufs=2, space="PSUM") as ppv,
            ):
                def out_proj(mq):
                    # out[mq] = hidT^T @ WwT + bw (bias via rank-1 matmul)
                    po = pst.tile([128, 1024], f32, tag="st", name="po")
                    nc.tensor.matmul(po[:, 0:256],
                                     lhsT=ones[0:1, mq * 128:(mq + 1) * 128],
                                     rhs=bwb[0:1, :],
                                     start=True, stop=False)
                    for gg in range(2):
                        nc.tensor.matmul(
                            po[:, 0:256],
                            lhsT=hidT[gg][:, mq * 128:(mq + 1) * 128],
                            rhs=WwTt[gg],
                            start=False, stop=(gg == 1))
                    pcopy(outsb[:, mq * H:(mq + 1) * H], po[:, 0:256])
                    if mq % 2 == 1:
                        cs2 = slice(mq - 1, mq + 1)
                        nc.sync.dma_start(
                            out=out_d.rearrange("(c p) e -> p c e",
                                                p=128)[:, cs2, :],
                            in_=outsb.rearrange("p (c e) -> p c e",
                                                c=8)[:, cs2, :])

                for qh in range(2):
                    for g in range(4):
                        ch = g // 2
                        pv = ppv.tile([128, 512], f32, tag="pv",
                                      name=f"pv{qh}_{g}")
                        for kcp in range(8):
                            idx = (qh * 4 + g) * 8 + kcp
                            kcs = (2 * kcp, 2 * kcp + 1)
                            sts2 = {}
                            for kc2 in kcs:
                                sts2[kc2] = pst.tile([128, 1024], f32,
                                                     tag="st",
                                                     name=f"st{kc2 % 2}")
                            for kc2 in kcs:
                                par = kc2 % 2
                                kket = keffT[ch] if par == 0 else keffT2[ch]
                                qqt = qsT[ch] if par == 0 else qsT2[ch]
                                rbase = ((g % 2) * 64 if par == 0
                                         else (1 - g % 2) * 64)
                                for t in range(2):
                                    ro = rbase + t * 32
                                    nc.tensor.matmul(
                                        sts2[kc2][:, t * 512:(t + 1) * 512],
                                        lhsT=kket[ro:ro + 32,
                                                  kc2 * 128:(kc2 + 1) * 128],
                                        rhs=qqt[ro:ro + 32,
                                                qh * 512:(qh + 1) * 512],
                                        tile_position=(ro, 0),
                                        start=True, stop=True)
                            # exp: second tile of pair -> ACT; first -> DVE
                            # (Schraudolph), except every 8th pair both ACT
                            # (balance: 72 ACT / 56 DVE)
                            pts = {}
                            for kc2 in kcs:
                                if kc2 != kcs[0] or idx % 8 == 3:
                                    pt = ptp.tile([128, 1024], bf16,
                                                  tag="ptA", name="ptA")
                                    nc.scalar.activation(pt, sts2[kc2], Exp)
                                    pts[kc2] = pt
                                else:
                                    pti = ptp.tile([128, 1024], i16,
                                                   tag="ptD", name="ptD")
                                    nc.vector.tensor_scalar(
                                        out=pti, in0=sts2[kc2], scalar1=A16,
                                        scalar2=B16,
                                        op0=mybir.AluOpType.mult,
                                        op1=mybir.AluOpType.add)
                                    pts[kc2] = pti.bitcast(bf16)
                            # PV: h even lhsT=[m|v] -> rows [den|O];
                            #     h odd  lhsT=[v|m] -> rows [O|den]
                            for kc2 in kcs:
                                for t in range(2):
                                    h = 2 * g + t
                                    nc.tensor.matmul(
                                        pv[64 * t:64 * t + 64, :],
                                        lhsT=vmm[:, (kc2 * NH + h) * 64:
                                                 (kc2 * NH + h) * 64 + 64],
                                        rhs=pts[kc2][:,
                                                     t * 512:(t + 1) * 512],
                                        tile_position=(0, 64 * t),
                                        start=(kc2 == 0), stop=(kc2 == 15),
                                        skip_group_check=True)
                            # interleave qh=0's output projection into
                            # qh=1/g=0 so only mq 4-7 remain in the tail
                            if qh == 1 and g == 0 and 2 <= kcp <= 5:
                                out_proj(kcp - 2)
                        # normalize: pv rows = [den0 | O0 | O1 | den1].
                        # Full 128-partition ops; unused lanes compute
                        # garbage, unread.
                        ntmp = smp.tile([128, 512], f32, tag="ntmp",
                                        name="ntmp")
                        nc.vector.reciprocal_approx_fast(ntmp, pv)
                        rtl = smp.tile([128, 512], f32, tag="rtl", name="rtl")
                        # rows 0:32 / 96:128 are dummy-inits (lanes unread)
                        nc.sync.dma_start(out=rtl[0:32], in_=ntmp[0:32])
                        nc.sync.dma_start(out=rtl[32:64], in_=ntmp[0:32])
                        nc.sync.dma_start(out=rtl[64:96], in_=ntmp[96:128])
                        nc.sync.dma_start(out=rtl[96:128], in_=ntmp[96:128])
                        hst = smp.tile([128, 512], bf16, tag="hst",
                                       name="hst")
                        nc.vector.tensor_mul(hst, pv, rtl)
                        ro2 = (g % 2) * 64
                        nc.sync.dma_start(
                            out=hidT[ch][ro2:ro2 + 64,
                                         qh * 512:(qh + 1) * 512],
                            in_=hst[32:96])
                # tail: remaining output projection
                for mq in range(4, 8):
                    out_proj(mq)

    nc.compile()
    return nc


def _make_in_maps(inputs):
    import ml_dtypes

    bf16 = ml_dtypes.bfloat16
    q = np.asarray(inputs["q"], dtype=np.float32)
    k = np.asarray(inputs["k"], dtype=np.float32)
    v = np.asarray(inputs["v"], dtype=np.float32)
    k_b = np.asarray(inputs["k_b"], dtype=np.float32)
    mask = np.asarray(inputs["mask"], dtype=np.int32)
    sw = np.asarray(inputs["scale_w"], dtype=np.float32)
    Wb = np.asarray(inputs["Wb"], dtype=np.float32)
    bb = np.asarray(inputs["bb"], dtype=np.float32)
    Ww = np.asarray(inputs["Ww"], dtype=np.float32)
    bw = np.asarray(inputs["bw"], dtype=np.float32)

    WbT = np.ascontiguousarray(Wb.T).astype(bf16)
    WwT = np.ascontiguousarray(Ww.T).astype(bf16)
    bbb = bb[None, :].astype(bf16)
    bwb = bw[None, :].astype(bf16)
    ones = np.ones((1, L), dtype=bf16)

    per_batch = {}
    for b in range(B):
        kT = np.ascontiguousarray(k[b].T).astype(bf16)
        kbT = np.ascontiguousarray(k_b[b].T).astype(bf16)
        # vmm: [128, kc(16) x h(8) x two(2) x d(32)]
        # h even: [mask | v*mask];  h odd: [v*mask | mask]
        v4 = v[b].reshape(16, 128, NH, DH)
        mk = mask[b].reshape(16, 128).astype(np.float32)
        vm = v4 * mk[:, :, None, None]
        vmm = np.empty((16, 128, NH, 2, DH), dtype=np.float32)
        for h in range(NH):
            vmm[:, :, h, 1 - h % 2, :] = vm[:, :, h, :]
            vmm[:, :, h, h % 2, :] = mk[:, :, None]
        vmm = np.ascontiguousarray(
            vmm.transpose(1, 0, 2, 3, 4).reshape(128, 8192)).astype(bf16)
        per_batch[b] = (kT, kbT, vmm)

    in_maps = []
    for c in range(NCORES):
        b, qs = c // 2, c % 2
        kT, kbT, vmm = per_batch[b]
        qc = q[b, qs * LQ:(qs + 1) * LQ, :]  # [1024, 256]
        q16 = np.ascontiguousarray(
            qc.reshape(8, 128, H).transpose(1, 0, 2).reshape(128, 2048)
        ).astype(bf16)
        # sc8: [128 qpart, chunk(8) x head(8)] = scale_w * 1/sqrt(DH)
        swc = sw[:, qs * LQ:(qs + 1) * LQ] * ISQ  # [8, 1024]
        sc8 = np.ascontiguousarray(
            swc.reshape(NH, 8, 128).transpose(2, 1, 0).reshape(128, 64))
        in_maps.append({
            "q16": q16, "kT": kT, "kbT": kbT, "vmm": vmm, "sc8": sc8,
            "WbT": WbT, "WwT": WwT, "bbb": bbb, "bwb": bwb, "ones": ones,
        })
    return in_maps


def run_sharded(inputs, trace=False, tmpdir=None):
    from concourse import bass_utils
    from concourse.bass_utils import run_bass_kernel_spmd

    if trace:
        _install_ntff_hook()
        bass_utils.upload_artifacts = lambda d: d
    nc = _build()
    in_maps = _make_in_maps(inputs)
    res = run_bass_kernel_spmd(nc, in_maps, list(range(NCORES)),
                               trace=trace, tmpdir=tmpdir)
    out = np.empty((B, L, H), dtype=np.float32)
    for c in range(NCORES):
        b, qs = c // 2, c % 2
        out[b, qs * LQ:(qs + 1) * LQ, :] = res.results[c]["out"]
    return out, res


def kernel(**inputs):
    out, _ = run_sharded(inputs, trace=False)
    return out


def _install_ntff_hook():
    """Provide antenv.axon_hooks (absent in this image) so trace=True works."""
    import contextlib
    import ctypes
    import types

    import antenv

    if hasattr(antenv, "axon_hooks"):
        return
    mod = types.ModuleType("antenv.axon_hooks")
    _hook = [None]
    mod.set_axon_ntff_profile_hook = lambda h: _hook.__setitem__(0, h)
    mod.get_axon_ntff_profile_hook = lambda: _hook[0]
    antenv.axon_hooks = mod
    sys.modules["antenv.axon_hooks"] = mod

    lib = ctypes.CDLL("/opt/axon/libaxon_pjrt.so")
    if not hasattr(lib, "axon_start_nrt_profile"):
        return
    lib.axon_start_nrt_profile.argtypes = [ctypes.POINTER(ctypes.c_int64),
                                           ctypes.c_size_t]
    lib.axon_start_nrt_profile.restype = ctypes.c_int64
    lib.axon_stop_nrt_profile.argtypes = [ctypes.c_char_p]
    lib.axon_stop_nrt_profile.restype = ctypes.c_int64

    @contextlib.contextmanager
    def _profile(output_dir, device_ids):
        import jax

        jax.devices()
        if device_ids:
            ids = (ctypes.c_int64 * len(device_ids))(*device_ids)
            rc = lib.axon_start_nrt_profile(ids, len(device_ids))
        else:
            rc = lib.axon_start_nrt_profile(None, 0)
        if rc != 0:
            raise RuntimeError(f"axon_start_nrt_profile rc={rc}")
        try:
            yield
        finally:
            n = lib.axon_stop_nrt_profile(str(output_dir).encode())
            print(f"profile: {n} file(s) written to {output_dir}",
                  file=sys.stderr)

    mod.set_axon_ntff_profile_hook(_profile)


# revision 7
# speedup vs baseline: 1.0104x; 1.0104x over previous
"""Trainium2 Bass kernel for AuxiliaryMultiHeadedAttention.

Reference computation (B=4, L=2048, H=256, NH=8, DH=32):
    kb   = split_heads(k_b @ Wb.T + bb)
    corr = (qh @ kh^T + qh @ kb^T) / sqrt(DH) * scale_w[h, q]
    corr = where(mask==0, -1e9, corr);  prob = softmax(corr)
    out  = merge_heads(prob @ vh) @ Ww.T + bw

Kernel strategy (8 NeuronCores):
    Shard (batch, query-half): core c -> batch c//2, queries (c%2)*1024..+1024.
    Host marshals layouts (pre-transposed bf16 k^T / k_b^T / W^T, the
    mask-interleaved V operand, scale_w chunks) so the device does zero
    layout shuffling; device computes both GEMMs, QK^T, softmax, PV and
    the output projection.
    Each core:
      keffT = (k + k_b @ Wb.T + bb)^T  [dims, keys]  bf16 (dual QK^T folded)
      qsT   = (q * scale_w/sqrt(DH))^T via DMA-xbar  [dims, queries] bf16
      S^T   = keffT_h^T @ qsT_h  (bf16 MMs, 2 heads row-tiled, fp32 psum;
              kc pairs alternate PE row groups via 64-row-shifted copies)
      P^T   = exp(S^T): split between ACT (exact exp, bf16 out) and DVE
              (Schraudolph: int16(A*x+B) bitcast to bf16, one tensor_scalar)
      PV with lhsT [m|v_h] / [v_h|m] (m = mask: masks numerator and
              denominator) -> psum rows [den0|O0|O1|den1]
      hidT  = O * recip(den)  (full-partition recip + mul, DMA realign)
      out   = hidT^T @ WwT + bw (bias via rank-1 matmul), query-half-outer
              loop lets the first half's projection overlap the second half.
    Host concatenates the 8 [1024, 256] slices.
"""

import sys

if "/opt/trn_rl_repo" not in sys.path:
    sys.path.insert(0, "/opt/trn_rl_repo")

import math

import numpy as np

B, L, H, NH, DH = 4, 2048, 256, 8, 32
LQ = 1024  # queries per core
NCORES = 8
ISQ = 1.0 / math.sqrt(DH)

# Schraudolph exp for bf16 target: bf16bits(exp(x)) ~ int16(A16*x + B16)
A16 = 128.0 / math.log(2.0)
C_OFF = 5.5
B16 = 127.0 * 128.0 - C_OFF


def _build():
    import concourse.bass as bass  # noqa: F401
    import concourse.mybir as mybir
    import concourse.tile as tile
    from concourse import bacc

    f32 = mybir.dt.float32
    i16 = mybir.dt.int16
    bf16 = mybir.dt.bfloat16
    Exp = mybir.ActivationFunctionType.Exp

    nc = bacc.Bacc("TRN2", target_bir_lowering=False, debug=False, num_devices=NCORES)

    q16_d = nc.dram_tensor("q16", [128, 2048], bf16, kind="ExternalInput")
    kT_d = nc.dram_tensor("kT", [H, L], bf16, kind="ExternalInput")
    kbT_d = nc.dram_tensor("kbT", [H, L], bf16, kind="ExternalInput")
    vmm_d = nc.dram_tensor("vmm", [128, 8192], bf16, kind="ExternalInput")
    sc8_d = nc.dram_tensor("sc8", [128, 64], f32, kind="ExternalInput")
    WbT_d = nc.dram_tensor("WbT", [H, H], bf16, kind="ExternalInput")
    WwT_d = nc.dram_tensor("WwT", [H, H], bf16, kind="ExternalInput")
    bwb_d = nc.dram_tensor("bwb", [1, H], bf16, kind="ExternalInput")
    ones_d = nc.dram_tensor("ones", [1, L], bf16, kind="ExternalInput")
    out_d = nc.dram_tensor("out", [LQ, H], f32, kind="ExternalOutput")

    copy_flip = [0]

    with tile.TileContext(nc) as tc:
        with (
            tc.tile_pool(name="persist", bufs=1) as pp,
            tc.tile_pool(name="pt", bufs=3) as ptp,
            tc.tile_pool(name="small", bufs=2) as smp,
        ):
            # ---------------- persistent SBUF tensors ----------------
            keffT = [pp.tile([128, L], bf16, tag=f"keffT{g}", name=f"keffT{g}")
                     for g in range(2)]
            keffT2 = [pp.tile([128, L], bf16, tag=f"keffT2_{g}",
                              name=f"keffT2_{g}") for g in range(2)]
            qsT = [pp.tile([128, LQ], bf16, tag=f"qsT{g}", name=f"qsT{g}")
                   for g in range(2)]
            qsT2 = [pp.tile([128, LQ], bf16, tag=f"qsT2_{g}", name=f"qsT2_{g}")
                    for g in range(2)]
            # per (key-chunk, head): [m|v_h] (h even) / [v_h|m] (h odd);
            # m = mask column (masks numerator and denominator)
            vmm = pp.tile([128, 16 * NH * 64], bf16, tag="vmm")
            hidT = [pp.tile([128, LQ], bf16, tag=f"hidT{g}", name=f"hidT{g}")
                    for g in range(2)]
            outsb = pp.tile([128, 8 * H], f32, tag="outsb")

            kbTt = [pp.tile([128, L], bf16, tag=f"kbTt{e}", name=f"kbTt{e}")
                    for e in range(2)]
            kTt = [pp.tile([128, L], bf16, tag=f"kTt{e}", name=f"kTt{e}")
                   for e in range(2)]
            qb16 = pp.tile([128, 2048], bf16, tag="qb16")
            qs16 = [pp.tile([128, 1024], bf16, tag=f"qs16_{d}",
                            name=f"qs16_{d}") for d in range(2)]
            sc8 = pp.tile([128, 64], f32, tag="sc8")
            WbTt = [pp.tile([128, H], bf16, tag=f"WbTt{e}", name=f"WbTt{e}")
                    for e in range(2)]
            WwTt = [pp.tile([128, H], bf16, tag=f"WwTt{g}", name=f"WwTt{g}")
                    for g in range(2)]
            bwb = pp.tile([1, H], bf16, tag="bwb")
            ones = pp.tile([1, L], bf16, tag="ones")

            def pcopy(dst, src):
                # alternate psum->sbuf evacuation between DVE and ACT
                if copy_flip[0] % 2 == 0:
                    nc.vector.tensor_copy(dst, src)
                else:
                    nc.scalar.copy(dst, src)
                copy_flip[0] += 1

            # ---------------- staging loads ----------------
            # 3 DMA queues (sync/scalar HWDGE + gpsimd SWDGE), ordered by
            # criticality; vmm (largest) is chunked so it trails into the
            # main loop (PV consumes it in kc order).
            vmv = vmm.rearrange("p (c f) -> p c f", c=4)
            # gpsimd: kbT (keff critical path), then vmm kc 8-15
            for ec in range(2):
                nc.gpsimd.dma_start(out=kbTt[ec],
                                    in_=kbT_d[ec * 128:(ec + 1) * 128, :])
            nc.gpsimd.dma_start(out=vmv[:, 2, :], in_=vmm_d[:, 4096:6144])
            nc.gpsimd.dma_start(out=vmv[:, 3, :], in_=vmm_d[:, 6144:8192])
            # sync: keff/q deps first, tail-only tensors last
            nc.sync.dma_start(out=WbTt[0], in_=WbT_d[0:128, :])
            nc.sync.dma_start(out=WbTt[1], in_=WbT_d[128:256, :])
            nc.sync.dma_start(out=sc8, in_=sc8_d[:, :])
            nc.sync.dma_start(out=qb16, in_=q16_d[:, :])
            for ec in range(2):
                nc.sync.dma_start(out=kTt[ec],
                                  in_=kT_d[ec * 128:(ec + 1) * 128, :])
            # scalar: vmm kc 0-3, q xbars + 64-row shifts, vmm kc 4-7
            nc.scalar.dma_start(out=vmv[:, 0, :], in_=vmm_d[:, 0:2048])

            # q scale: qs16[dc] = q_bf16 * (scale_w * ISQ)  [per head, query]
            sc8r = sc8.rearrange("p (c h) -> p c h", c=8)
            qv = qb16.rearrange("p (c e) -> p c e", c=8)
            for dc in range(2):
                nc.vector.tensor_mul(
                    qs16[dc].rearrange("p (c h j) -> p c h j", c=8, h=4),
                    qv[:, :, dc * 128:(dc + 1) * 128].rearrange(
                        "p c (h j) -> p c h j", h=4),
                    sc8r[:, :, dc * 4:(dc + 1) * 4][:, :, :, None].broadcast_to(
                        [128, 8, 4, 32]))
                nc.scalar.dma_start_transpose(
                    out=qsT[dc].rearrange("p (c f) -> p c f", c=8),
                    in_=qs16[dc])
                nc.scalar.dma_start(out=qsT2[dc][0:64], in_=qsT[dc][64:128])
                nc.scalar.dma_start(out=qsT2[dc][64:128], in_=qsT[dc][0:64])
            nc.scalar.dma_start(out=vmv[:, 1, :], in_=vmm_d[:, 2048:4096])
            nc.sync.dma_start(out=WwTt[0], in_=WwT_d[0:128, :])
            nc.sync.dma_start(out=WwTt[1], in_=WwT_d[128:256, :])
            nc.sync.dma_start(out=bwb, in_=bwb_d[:, :])
            nc.sync.dma_start(out=ones, in_=ones_d[:, :])

            # ---------------- keff GEMM ----------------
            # bb is pre-added into kT on the host, so no bias matmuls:
            # keffT = WbT.T @ kbT (psum) + (k.T + bb) via fused evac add.
            with tc.tile_pool(name="pkeff", bufs=1, space="PSUM") as pkf:
                for dc in range(2):
                    pk = pkf.tile([128, L], f32, tag="pk", name=f"pk{dc}")
                    for ec in range(2):
                        for ns in range(4):
                            nc.tensor.matmul(
                                pk[:, ns * 512:(ns + 1) * 512],
                                lhsT=WbTt[ec][:, dc * 128:(dc + 1) * 128],
                                rhs=kbTt[ec][:, ns * 512:(ns + 1) * 512],
                                start=(ec == 0), stop=(ec == 1))
                    # evacuate with fused +(k+bb) add; then 64-row shift
                    for nh2 in range(2):
                        co = slice(nh2 * 1024, (nh2 + 1) * 1024)
                        nc.vector.tensor_add(keffT[dc][:, co], pk[:, co],
                                             kTt[dc][:, co])
                    nc.sync.dma_start(out=keffT2[dc][0:64],
                                      in_=keffT[dc][64:128])
                    nc.sync.dma_start(out=keffT2[dc][64:128],
                                      in_=keffT[dc][0:64])

            # ---------------- main attention loop ----------------
            # query-half outer so the first half's output projection can
            # overlap the second half's attention.
            # group g: heads (2g, 2g+1); chunk ch = g//2.
            # kc processed in pairs with alternating PE row groups (via the
            # 64-row-shifted tile copies): the pair's 4 QK matmuls occupy 4
            # distinct 32-row groups and stream concurrently.
            with (
                tc.tile_pool(name="pst", bufs=3, space="PSUM") as pst,
                tc.tile_pool(name="ppv", bufs=2, space="PSUM") as ppv,
            ):
                def out_proj(mq):
                    # out[mq] = hidT^T @ WwT + bw (bias via rank-1 matmul)
                    po = pst.tile([128, 1024], f32, tag="st", name="po")
                    nc.tensor.matmul(po[:, 0:256],
                                     lhsT=ones[0:1, mq * 128:(mq + 1) * 128],
                                     rhs=bwb[0:1, :],
                                     start=True, stop=False)
                    for gg in range(2):
                        nc.tensor.matmul(
                            po[:, 0:256],
                            lhsT=hidT[gg][:, mq * 128:(mq + 1) * 128],
                            rhs=WwTt[gg],
                            start=False, stop=(gg == 1))
                    pcopy(outsb[:, mq * H:(mq + 1) * H], po[:, 0:256])
                    if mq % 2 == 1:
                        cs2 = slice(mq - 1, mq + 1)
                        nc.sync.dma_start(
                            out=out_d.rearrange("(c p) e -> p c e",
                                                p=128)[:, cs2, :],
                            in_=outsb.rearrange("p (c e) -> p c e",
                                                c=8)[:, cs2, :])

                for qh in range(2):
                    for g in range(4):
                        ch = g // 2
                        pv = ppv.tile([128, 512], f32, tag="pv",
                                      name=f"pv{qh}_{g}")
                        for kcp in range(8):
                            idx = (qh * 4 + g) * 8 + kcp
                            kcs = (2 * kcp, 2 * kcp + 1)
                            sts2 = {}
                            for kc2 in kcs:
                                sts2[kc2] = pst.tile([128, 1024], f32,
                                                     tag="st",
                                                     name=f"st{kc2 % 2}")
                            for kc2 in kcs:
                                par = kc2 % 2
                                kket = keffT[ch] if par == 0 else keffT2[ch]
                                qqt = qsT[ch] if par == 0 else qsT2[ch]
                                rbase = ((g % 2) * 64 if par == 0
                                         else (1 - g % 2) * 64)
                                for t in range(2):
                                    ro = rbase + t * 32
                                    nc.tensor.matmul(
                                        sts2[kc2][:, t * 512:(t + 1) * 512],
                                        lhsT=kket[ro:ro + 32,
                                                  kc2 * 128:(kc2 + 1) * 128],
                                        rhs=qqt[ro:ro + 32,
                                                qh * 512:(qh + 1) * 512],
                                        tile_position=(ro, 0),
                                        start=True, stop=True)
                            # exp: second tile of pair -> ACT; first -> DVE
                            # (Schraudolph), except every 8th pair both ACT
                            # (balance: 72 ACT / 56 DVE)
                            pts = {}
                            for kc2 in kcs:
                                if kc2 != kcs[0] or idx % 8 == 3:
                                    pt = ptp.tile([128, 1024], bf16,
                                                  tag="ptA", name="ptA")
                                    nc.scalar.activation(pt, sts2[kc2], Exp)
                                    pts[kc2] = pt
                                else:
                                    pti = ptp.tile([128, 1024], i16,
                                                   tag="ptD", name="ptD")
                                    nc.vector.tensor_scalar(
                                        out=pti, in0=sts2[kc2], scalar1=A16,
                                        scalar2=B16,
                                        op0=mybir.AluOpType.mult,
                                        op1=mybir.AluOpType.add)
                                    pts[kc2] = pti.bitcast(bf16)
                            # PV: h even lhsT=[m|v] -> rows [den|O];
                            #     h odd  lhsT=[v|m] -> rows [O|den]
                            for kc2 in kcs:
                                for t in range(2):
                                    h = 2 * g + t
                                    nc.tensor.matmul(
                                        pv[64 * t:64 * t + 64, :],
                                        lhsT=vmm[:, (kc2 * NH + h) * 64:
                                                 (kc2 * NH + h) * 64 + 64],
                                        rhs=pts[kc2][:,
                                                     t * 512:(t + 1) * 512],
                                        tile_position=(0, 64 * t),
                                        start=(kc2 == 0), stop=(kc2 == 15),
                                        skip_group_check=True)
                            # interleave qh=0's output projection into
                            # qh=1/g=0 so only mq 4-7 remain in the tail
                            if qh == 1 and g == 0 and 2 <= kcp <= 5:
                                out_proj(kcp - 2)
                        # normalize: pv rows = [den0 | O0 | O1 | den1].
                        # Full 128-partition ops; unused lanes compute
                        # garbage, unread.
                        ntmp = smp.tile([128, 512], f32, tag="ntmp",
                                        name="ntmp")
                        nc.vector.reciprocal_approx_fast(ntmp, pv)
                        rtl = smp.tile([128, 512], f32, tag="rtl", name="rtl")
                        # rows 0:32 / 96:128 are dummy-inits (lanes unread)
                        nc.sync.dma_start(out=rtl[0:32], in_=ntmp[0:32])
                        nc.sync.dma_start(out=rtl[32:64], in_=ntmp[0:32])
                        nc.sync.dma_start(out=rtl[64:96], in_=ntmp[96:128])
                        nc.sync.dma_start(out=rtl[96:128], in_=ntmp[96:128])
                        hst = smp.tile([128, 512], bf16, tag="hst",
                                       name="hst")
                        nc.vector.tensor_mul(hst, pv, rtl)
                        ro2 = (g % 2) * 64
                        nc.sync.dma_start(
                            out=hidT[ch][ro2:ro2 + 64,
                                         qh * 512:(qh + 1) * 512],
                            in_=hst[32:96])
                # tail: remaining output projection
                for mq in range(4, 8):
                    out_proj(mq)

    nc.compile()
    return nc


def _make_in_maps(inputs):
    import ml_dtypes

    bf16 = ml_dtypes.bfloat16
    q = np.asarray(inputs["q"], dtype=np.float32)
    k = np.asarray(inputs["k"], dtype=np.float32)
    v = np.asarray(inputs["v"], dtype=np.float32)
    k_b = np.asarray(inputs["k_b"], dtype=np.float32)
    mask = np.asarray(inputs["mask"], dtype=np.int32)
    sw = np.asarray(inputs["scale_w"], dtype=np.float32)
    Wb = np.asarray(inputs["Wb"], dtype=np.float32)
    bb = np.asarray(inputs["bb"], dtype=np.float32)
    Ww = np.asarray(inputs["Ww"], dtype=np.float32)
    bw = np.asarray(inputs["bw"], dtype=np.float32)

    WbT = np.ascontiguousarray(Wb.T).astype(bf16)
    WwT = np.ascontiguousarray(Ww.T).astype(bf16)
    bwb = bw[None, :].astype(bf16)
    ones = np.ones((1, L), dtype=bf16)

    per_batch = {}
    for b in range(B):
        kT = np.ascontiguousarray(k[b].T + bb[:, None]).astype(bf16)
        kbT = np.ascontiguousarray(k_b[b].T).astype(bf16)
        # vmm: [128, kc(16) x h(8) x two(2) x d(32)]
        # h even: [mask | v*mask];  h odd: [v*mask | mask]
        v4 = v[b].reshape(16, 128, NH, DH)
        mk = mask[b].reshape(16, 128).astype(np.float32)
        vm = v4 * mk[:, :, None, None]
        vmm = np.empty((16, 128, NH, 2, DH), dtype=np.float32)
        for h in range(NH):
            vmm[:, :, h, 1 - h % 2, :] = vm[:, :, h, :]
            vmm[:, :, h, h % 2, :] = mk[:, :, None]
        vmm = np.ascontiguousarray(
            vmm.transpose(1, 0, 2, 3, 4).reshape(128, 8192)).astype(bf16)
        per_batch[b] = (kT, kbT, vmm)

    in_maps = []
    for c in range(NCORES):
        b, qs = c // 2, c % 2
        kT, kbT, vmm = per_batch[b]
        qc = q[b, qs * LQ:(qs + 1) * LQ, :]  # [1024, 256]
        q16 = np.ascontiguousarray(
            qc.reshape(8, 128, H).transpose(1, 0, 2).reshape(128, 2048)
        ).astype(bf16)
        # sc8: [128 qpart, chunk(8) x head(8)] = scale_w * 1/sqrt(DH)
        swc = sw[:, qs * LQ:(qs + 1) * LQ] * ISQ  # [8, 1024]
        sc8 = np.ascontiguousarray(
            swc.reshape(NH, 8, 128).transpose(2, 1, 0).reshape(128, 64))
        in_maps.append({
            "q16": q16, "kT": kT, "kbT": kbT, "vmm": vmm, "sc8": sc8,
            "WbT": WbT, "WwT": WwT, "bwb": bwb, "ones": ones,
        })
    return in_maps


def run_sharded(inputs, trace=False, tmpdir=None):
    from concourse import bass_utils
    from concourse.bass_utils import run_bass_kernel_spmd

    if trace:
        _install_ntff_hook()
        bass_utils.upload_artifacts = lambda d: d
    nc = _build()
    in_maps = _make_in_maps(inputs)
    res = run_bass_kernel_spmd(nc, in_maps, list(range(NCORES)),
                               trace=trace, tmpdir=tmpdir)
    out = np.empty((B, L, H), dtype=np.float32)
    for c in range(NCORES):
        b, qs = c // 2, c % 2
        out[b, qs * LQ:(qs + 1) * LQ, :] = res.results[c]["out"]
    return out, res


def kernel(**inputs):
    out, _ = run_sharded(inputs, trace=False)
    return out


def _install_ntff_hook():
    """Provide antenv.axon_hooks (absent in this image) so trace=True works."""
    import contextlib
    import ctypes
    import types

    import antenv

    if hasattr(antenv, "axon_hooks"):
        return
    mod = types.ModuleType("antenv.axon_hooks")
    _hook = [None]
    mod.set_axon_ntff_profile_hook = lambda h: _hook.__setitem__(0, h)
    mod.get_axon_ntff_profile_hook = lambda: _hook[0]
    antenv.axon_hooks = mod
    sys.modules["antenv.axon_hooks"] = mod

    lib = ctypes.CDLL("/opt/axon/libaxon_pjrt.so")
    if not hasattr(lib, "axon_start_nrt_profile"):
        return
    lib.axon_start_nrt_profile.argtypes = [ctypes.POINTER(ctypes.c_int64),
                                           ctypes.c_size_t]
    lib.axon_start_nrt_profile.restype = ctypes.c_int64
    lib.axon_stop_nrt_profile.argtypes = [ctypes.c_char_p]
    lib.axon_stop_nrt_profile.restype = ctypes.c_int64

    @contextlib.contextmanager
    def _profile(output_dir, device_ids):
        import jax

        jax.devices()
        if device_ids:
            ids = (ctypes.c_int64 * len(device_ids))(*device_ids)
            rc = lib.axon_start_nrt_profile(ids, len(device_ids))
        else:
            rc = lib.axon_start_nrt_profile(None, 0)
        if rc != 0:
            raise RuntimeError(f"axon_start_nrt_profile rc={rc}")
        try:
            yield
        finally:
            n = lib.axon_stop_nrt_profile(str(output_dir).encode())
            print(f"profile: {n} file(s) written to {output_dir}",
                  file=sys.stderr)

    mod.set_axon_ntff_profile_hook(_profile)
